# revision 38
# baseline (speedup 1.0000x reference)
"""Causal self-attention (B=4, L=2048, D=1024, H=16, HD=64) on 8 TRN2 cores.

Sharding: 8 shards = 4 batches x 2 head-groups (8 heads each). Each core:
  - QKV projection for its 8 heads (Q^T/K^T in [HD, L] layout, V in [L, HD])
  - causal attention per head, softmax without max-subtraction (logits are
    small by construction), row sums via a ones-column appended to V
  - partial output projection with its 512 rows of out_w
Host sums the two partials per batch and adds out_b.

v3 structure (vs v2): the attention inner loop is software-pipelined (S
matmuls run one key-block ahead of the AV matmuls, so the PE never waits
on ScalarE's exp); the U accumulator is copied PSUM->SBUF right after
its last AV so the single PSUM accumulator frees quickly, and the
normalize chain (reciprocal -> PE broadcast -> scale straight from
PSUM) runs off the critical path, injected into the next head-pair's
stream; projection and output-projection matmul groups are interleaved
as "filler" into attention's PE slack.

Engine placement is tuned to measured hardware (not the cost model):
DVE instruction count is kept minimal (its effective per-op cost under
load is ~1us); PSUM->SBUF drains go to ScalarE, the causal-mask
multiply (SBUF-only) goes to Pool/gpsimd, which cannot touch PSUM.

All matmuls run in bf16 (fp32 PSUM accumulation); exp on ScalarE in
fp32. An fp8e4 DoubleRow path for the projections exists behind
KERNEL_FP8_QK/KERNEL_FP8_V (off: ~3e-2 rel err, no HW speedup).
"""

import os
from collections import deque

import numpy as np
import ml_dtypes

B, L, D, H, HD = 4, 2048, 1024, 16, 64
HPC = 8           # heads per core
NCORES = 8
QT_TILE = 512     # q columns per attention tile
NKB = L // 128    # key blocks of 128
NJT = L // QT_TILE

_STATE = {}


def _build_nc(repeat=1):
    import concourse.bass as bass
    import concourse.mybir as mybir
    import concourse.tile as tile
    from concourse import bacc
    from concourse.masks import make_upper_triangular

    f32 = mybir.dt.float32
    bf16 = mybir.dt.bfloat16
    fp8 = mybir.dt.float8e4
    AF = mybir.ActivationFunctionType
    OP = mybir.AluOpType
    DR = mybir.MatmulPerfMode.DoubleRow

    fp8_qk = bool(int(os.environ.get("KERNEL_FP8_QK", "0")))
    fp8_v = bool(int(os.environ.get("KERNEL_FP8_V", "0")))
    use_fp8 = fp8_qk or fp8_v  # x (shared operand) is fp8 if either path is
    in_dt = fp8 if use_fp8 else bf16
    # fp8 path: host scales wqk/wv (and their biases) by 32 to keep the
    # weights in e4m3's normal range. The 32x on q and k cancels via the
    # exp scale (0.125/32^2); the 32x on v cancels via w2/32 on the host.
    exp_scale = 0.125 / 1024.0 if fp8_qk else 0.125

    nc = bacc.Bacc(None, target_bir_lowering=False)

    qk_dt = fp8 if fp8_qk else bf16
    v_dt = fp8 if fp8_v else bf16
    xT = nc.dram_tensor("xT", [D, L], in_dt, kind="ExternalInput")
    wqk = nc.dram_tensor("wqk", [D, 2 * HPC * HD], qk_dt, kind="ExternalInput")
    wv = nc.dram_tensor("wv", [D, HPC * HD], v_dt, kind="ExternalInput")
    bqk = nc.dram_tensor("bqk", [128, 8], f32, kind="ExternalInput")
    bv = nc.dram_tensor("bv", [1, HPC * HD], bf16, kind="ExternalInput")
    w2 = nc.dram_tensor("w2", [HPC * HD, D], bf16, kind="ExternalInput")
    out = nc.dram_tensor("out", [L, D], bf16, kind="ExternalOutput")

    KO = D // 128  # contraction blocks for the projections
    FILL_EVERY = int(os.environ.get("KERNEL_FILL_EVERY", "2"))

    with tile.TileContext(nc) as tc:
        with (
            tc.tile_pool(name="const", bufs=1) as cpool,
            tc.tile_pool(name="weights", bufs=1) as wpool,
            tc.tile_pool(name="resident", bufs=1) as rpool,
            tc.tile_pool(name="xc", bufs=4) as xcpool,
            tc.tile_pool(name="ework", bufs=4) as epool,
            tc.tile_pool(name="usb", bufs=3) as upool,
            tc.tile_pool(name="ywork", bufs=3) as ypool,
            tc.tile_pool(name="rwork", bufs=3) as rwpool,
            tc.tile_pool(name="ps_mm", bufs=2, space="PSUM") as ps_mm,
            tc.tile_pool(name="ps_s", bufs=2, space="PSUM") as ps_s,
            tc.tile_pool(name="ps_u", bufs=1, space="PSUM") as ps_u,
        ):
            ones = cpool.tile([1, 128], bf16)
            nc.vector.memset(ones[:], 1.0)
            mask = cpool.tile([128, 128], bf16)
            make_upper_triangular(nc, mask[:], val=1.0, diag=True)
            # same mask replicated for the fused even/odd [128, 2, 128] op
            mask3 = cpool.tile([128, 2, 128], bf16)
            nc.vector.tensor_copy(mask3[:, 0, :], mask[:])
            nc.vector.tensor_copy(mask3[:, 1, :], mask[:])

            wqk_sb = wpool.tile([128, KO, 2 * HPC * HD], qk_dt)
            nc.sync.dma_start(wqk_sb[:], wqk.rearrange("(ko p) m -> p ko m", p=128))
            wv_sb = wpool.tile([128, KO, HPC * HD], v_dt)
            nc.sync.dma_start(wv_sb[:], wv.rearrange("(ko p) m -> p ko m", p=128))
            bqk_sb = wpool.tile([128, 8], f32)
            nc.sync.dma_start(bqk_sb[:], bqk[:])
            bv_sb = wpool.tile([1, HPC * HD], bf16)
            nc.sync.dma_start(bv_sb[:], bv[:])
            w2_sb = wpool.tile([128, 4, D], bf16)
            nc.sync.dma_start(w2_sb[:], w2.rearrange("(o p) n -> p o n", p=128))

            # Q^T / K^T packed as head pairs: head h lives at partitions
            # (h%2)*64..+64 of block h//2.
            QT = rpool.tile([128, 4, L], bf16)
            KT = rpool.tile([128, 4, L], bf16)
            # V with a ones column at index 64 (col 65 is alignment padding).
            V = rpool.tile([128, NKB, HPC, 66], bf16)
            nc.vector.memset(V[:, :, :, 64:66], 0.0)
            nc.vector.memset(V[:, :, :, 64:65], 1.0)
            OT = rpool.tile([128, 4, L], bf16)

            xTr = xT.rearrange("(ko p) n -> p ko n", p=128)

            def emit_pass():
                filler = deque()

                xcs = []
                for jt in range(NJT):
                    xc = xcpool.tile([128, KO, 512], in_dt, tag="xc")
                    nc.sync.dma_start(
                        xc[:], xTr[:, :, jt * 512:(jt + 1) * 512])
                    xcs.append(xc)

                def qk_thunk(jt, mb):
                    def f():
                        sl = slice(jt * 512, (jt + 1) * 512)
                        t = ps_mm.tile([128, 512], f32, tag="mm512")
                        if fp8_qk:
                            for kd in range(KO // 2):
                                nc.tensor.matmul(
                                    t[:],
                                    wqk_sb[:, 2 * kd:2 * kd + 2,
                                           mb * 128:(mb + 1) * 128],
                                    xcs[jt][:, 2 * kd:2 * kd + 2, :],
                                    start=(kd == 0),
                                    stop=(kd == KO // 2 - 1),
                                    perf_mode=DR,
                                )
                        else:
                            for ko in range(KO):
                                nc.tensor.matmul(
                                    t[:],
                                    wqk_sb[:, ko, mb * 128:(mb + 1) * 128],
                                    xcs[jt][:, ko, :],
                                    start=(ko == 0),
                                    stop=(ko == KO - 1),
                                )
                        dst = QT[:, mb, sl] if mb < 4 else KT[:, mb - 4, sl]
                        nc.vector.tensor_scalar_add(dst, t[:], bqk_sb[:, mb:mb + 1])
                    return f

                def v_thunk(jt, qb):
                    def f():
                        g = jt * 4 + qb
                        tv = ps_mm.tile([128, 512], f32, tag="mm512")
                        if fp8_v:
                            for kd in range(KO // 2):
                                nc.tensor.matmul(
                                    tv[:],
                                    xcs[jt][:, 2 * kd:2 * kd + 2,
                                            qb * 128:(qb + 1) * 128],
                                    wv_sb[:, 2 * kd:2 * kd + 2, :],
                                    start=(kd == 0),
                                    stop=False,
                                    perf_mode=DR,
                                )
                        else:
                            for ko in range(KO):
                                nc.tensor.matmul(
                                    tv[:],
                                    xcs[jt][:, ko, qb * 128:(qb + 1) * 128],
                                    wv_sb[:, ko, :],
                                    start=(ko == 0),
                                    stop=False,
                                )
                        nc.tensor.matmul(
                            tv[:], ones[0:1, :], bv_sb[0:1, :],
                            start=False, stop=True)
                        nc.scalar.copy(
                            V[:, g, :, 0:64],
                            tv.rearrange("p (h e) -> p h e", e=HD))
                    return f

                def outproj_thunk(qb, nb, copy_eng=None):
                    def f():
                        y_ps = ps_mm.tile([128, 512], f32, tag="mm512")
                        for hp4 in range(4):
                            nc.tensor.matmul(
                                y_ps[:],
                                OT[:, hp4, qb * 128:(qb + 1) * 128],
                                w2_sb[:, hp4, nb * 512:(nb + 1) * 512],
                                start=(hp4 == 0),
                                stop=(hp4 == 3),
                            )
                        y_sb = ypool.tile([128, 512], bf16, tag="y_sb")
                        if copy_eng == "scalar":
                            nc.scalar.copy(y_sb[:], y_ps[:])
                        else:
                            nc.vector.tensor_copy(y_sb[:], y_ps[:])
                        nc.sync.dma_start(
                            out[qb * 128:(qb + 1) * 128,
                                nb * 512:(nb + 1) * 512], y_sb[:])
                    return f

                def norm_b_thunk(jt, hp, usb, rcp):
                    def f():
                        sl = slice(jt * 512, (jt + 1) * 512)
                        # both reciprocal rows broadcast to partitions 0-63
                        # so the two scale multiplies have aligned SB inputs
                        b_ps_e = ps_mm.tile([64, 512], f32, tag="mm512")
                        b_ps_o = ps_mm.tile([64, 512], f32, tag="mm512")
                        nc.tensor.matmul(
                            b_ps_e[:], ones[0:1, 0:64], rcp[0:1, 0, :],
                            start=True, stop=True)
                        nc.tensor.matmul(
                            b_ps_o[:], ones[0:1, 0:64], rcp[0:1, 1, :],
                            start=True, stop=True)
                        nc.vector.tensor_tensor(
                            out=OT[0:64, hp, sl], in0=usb[0:64, 0, :],
                            in1=b_ps_e[:], op=OP.mult)
                        nc.vector.tensor_tensor(
                            out=OT[64:128, hp, sl], in0=usb[0:64, 1, :],
                            in1=b_ps_o[:], op=OP.mult)
                    return f

                blocks = [0]  # global attention block counter
                fill_every = [FILL_EVERY]

                def fill(n=None):
                    k = len(filler) if n is None else min(n, len(filler))
                    for _ in range(k):
                        filler.popleft()()

                def fill_cb():
                    blocks[0] += 1
                    if blocks[0] % fill_every[0] == 0:
                        fill(1)

                def attn_hp(jt, hp, inject=None):
                    nkb = (jt + 1) * (QT_TILE // 128)
                    u = ps_u.tile([65, 2, 512], f32, tag="u")
                    et_tiles = {}

                    def emit_S(kb):
                        q_off = max(0, kb * 128 - jt * QT_TILE)
                        qsl = slice(jt * QT_TILE + q_off, (jt + 1) * QT_TILE)
                        ksl = slice(kb * 128, (kb + 1) * 128)
                        s = ps_s.tile([128, 2, 512], f32, tag="s")
                        nc.tensor.matmul(
                            s[:, 0, q_off:], KT[0:64, hp, ksl],
                            QT[0:64, hp, qsl], start=True, stop=True)
                        nc.tensor.matmul(
                            s[:, 1, q_off:], KT[64:128, hp, ksl],
                            QT[64:128, hp, qsl], start=True, stop=True)
                        et = epool.tile([128, 2, 512], bf16, tag="et")
                        nc.scalar.activation(
                            et[:, :, q_off:], s[:, :, q_off:], AF.Exp,
                            scale=exp_scale)
                        if kb * 128 >= jt * QT_TILE:  # diagonal block
                            nc.gpsimd.tensor_tensor(
                                out=et[:, :, q_off:q_off + 128],
                                in0=et[:, :, q_off:q_off + 128],
                                in1=mask3[:],
                                op=OP.mult)
                        et_tiles[kb] = (et, q_off)

                    def emit_AV(kb):
                        et, q_off = et_tiles.pop(kb)
                        nc.tensor.matmul(
                            u[:, 0, q_off:], V[:, kb, 2 * hp, 0:65],
                            et[:, 0, q_off:],
                            start=(kb == 0), stop=(kb == nkb - 1))
                        nc.tensor.matmul(
                            u[:, 1, q_off:], V[:, kb, 2 * hp + 1, 0:65],
                            et[:, 1, q_off:],
                            start=(kb == 0), stop=(kb == nkb - 1))

                    emit_S(0)
                    for kb in range(1, nkb):
                        emit_S(kb)
                        emit_AV(kb - 1)
                        if inject is not None and kb in inject:
                            inject[kb]()
                        fill_cb()
                    emit_AV(nkb - 1)

                    # free the PSUM accumulators fast: even half copies out
                    # on DVE, odd half on Pool; reciprocal of the rowsums on
                    # DVE right away. The rest of the normalize (norm_b) is
                    # injected into the next stream once rcp is ready.
                    usb = upool.tile([65, 2, 512], bf16, tag="usb")
                    nc.vector.tensor_copy(usb[:], u[:])
                    rcp = rwpool.tile([1, 2, 512], bf16, tag="rcp")
                    with nc.allow_low_precision(
                            reason="bf16 recip feeds bf16 bcast"):
                        nc.vector.reciprocal(rcp[:], usb[64:65, :, :])
                    return norm_b_thunk(jt, hp, usb, rcp)

                phases = os.environ.get("KERNEL_PHASES", "ABC")

                # proj(0) runs un-overlapped (nothing to hide it in)
                if "A" in phases:
                    for mb in range(8):
                        qk_thunk(0, mb)()
                    for qb in range(4):
                        v_thunk(0, qb)()

                # outproj of jt fills attention of op_sched[jt] (late jts
                # have the longest, least-filled attention streams)
                op_sched = {0: 2, 1: 3, 2: 3}
                rr = ["scalar", "scalar"]
                pending_norm = None
                for jt in range(NJT):
                    if jt + 1 < NJT:
                        for mb in range(8):
                            filler.append(qk_thunk(jt + 1, mb))
                        for qb in range(4):
                            filler.append(v_thunk(jt + 1, qb))
                    for src, dst in op_sched.items():
                        if dst != jt:
                            continue
                        for qb in range(4 * src, 4 * src + 4):
                            for nb in range(D // 512):
                                filler.append(outproj_thunk(
                                    qb, nb, copy_eng=rr[(qb + nb) % 2]))
                    # spread the filler queue over this jt's whole stream
                    n_cb = 4 * (4 * (jt + 1) - 1)
                    fill_every[0] = max(FILL_EVERY,
                                        n_cb // max(1, len(filler) + 1))
                    for hp in range(4):
                        inject = {3: pending_norm} if pending_norm else None
                        pending_norm = attn_hp(jt, hp, inject)
                    fill()  # flush before next jt's attention needs QT/KT/V

                pending_norm()  # normalize for (NJT-1, hp3)
                rr3 = ["scalar", "scalar"]
                for qb in range(4 * (NJT - 1), 4 * NJT):
                    for nb in range(D // 512):
                        outproj_thunk(
                            qb, nb,
                            copy_eng=rr3[(2 * qb + nb) % 2])()

            for _rep in range(repeat):
                emit_pass()
    nc.compile()
    return nc


def _get_nc():
    if "nc" not in _STATE:
        _STATE["nc"] = _build_nc()
    return _STATE["nc"]


def kernel(x, in_w, in_b, out_w, out_b):
    from concourse.bass_utils import run_bass_kernel_spmd

    bf = ml_dtypes.bfloat16
    fp8_qk = bool(int(os.environ.get("KERNEL_FP8_QK", "0")))
    fp8_v = bool(int(os.environ.get("KERNEL_FP8_V", "0")))
    use_fp8 = fp8_qk or fp8_v
    in_np = ml_dtypes.float8_e4m3 if use_fp8 else bf
    qk_np = ml_dtypes.float8_e4m3 if fp8_qk else bf
    v_np = ml_dtypes.float8_e4m3 if fp8_v else bf
    ws_qk = 32.0 if fp8_qk else 1.0  # see _build_nc: folded back out
    ws_v = 32.0 if fp8_v else 1.0

    x = np.asarray(x, dtype=np.float32)
    in_w = np.asarray(in_w, dtype=np.float32)
    in_b = np.asarray(in_b, dtype=np.float32)
    out_w = np.asarray(out_w, dtype=np.float32)
    out_b = np.asarray(out_b, dtype=np.float32)

    nc = _get_nc()

    in_maps = []
    for c in range(NCORES):
        b, hg = c // 2, c % 2
        hsl = slice(hg * HPC * HD, (hg + 1) * HPC * HD)  # 512 cols of each section
        wq = in_w[:, 0:D][:, hsl] * ws_qk
        wk = in_w[:, D:2 * D][:, hsl] * ws_qk
        wv_ = in_w[:, 2 * D:3 * D][:, hsl] * ws_v
        bq = in_b[0:D][hsl] * ws_qk
        bk = in_b[D:2 * D][hsl] * ws_qk
        bv_ = in_b[2 * D:3 * D][hsl] * ws_v
        in_maps.append({
            "xT": np.ascontiguousarray(x[b].T).astype(in_np),
            "wqk": np.ascontiguousarray(
                np.concatenate([wq, wk], axis=1)).astype(qk_np),
            "wv": np.ascontiguousarray(wv_).astype(v_np),
            "bqk": np.ascontiguousarray(
                np.concatenate([bq, bk]).reshape(8, 128).T).astype(np.float32),
            "bv": np.ascontiguousarray(bv_.reshape(1, -1)).astype(bf),
            "w2": np.ascontiguousarray(out_w[hsl, :] / ws_v).astype(bf),
        })

    trace = bool(int(os.environ.get("KERNEL_TRACE", "0")))
    if not trace:
        # the axon NTFF profile hook is absent in this container; make sure a
        # stray BASS_TRACE=1 in the environment can't route us into it
        os.environ["BASS_NEVER_TRACE"] = "1"
    res = run_bass_kernel_spmd(
        nc, in_maps, core_ids=list(range(NCORES)), trace=trace,
    )
    _STATE["last_result"] = res
    _STATE["last_in_maps"] = in_maps

    y = np.zeros((B, L, D), dtype=np.float32)
    for c in range(NCORES):
        y[c // 2] += res.results[c]["out"].astype(np.float32)
    y += out_b[None, None, :]
    return y
